# revision 18
# baseline (speedup 1.0000x reference)
"""CQAttention (BiDAF context-query attention) Trainium2 Bass kernel.

Math (per batch b):
  Ct = C^T (Lc,d), Qt = Q^T (Lq,d), w = [w1,w2,w3]
  S[i,j]  = Ct[i].w1 + Qt[j].w2 + (Ct[i]*w3).Qt[j]
  S1      = softmax_j(S + qb[j])   (row softmax; per-i terms cancel)
  S2      = softmax_i(S + cb[i])   (col softmax; per-j terms cancel)
  A       = S1 @ Qt; T = S2^T @ Ct; Bmat = S1 @ T
  out     = concat([Ct, A, Ct*A, Ct*Bmat], -1)^T  -> (4d, Lc)

Single-exponential-grid design:
  E1^T[j,i] = exp(S^T + r2[j] + qb[j])  -- ONE exp pass (bf16), j on
  partitions, bias per-partition inside the ACT exp.
  S1 = E1 / rowsum_j(E1), and with u[i] = exp(r1[i] + cb[i]):
  S2 = (E1*u) / colsum_i(E1*u)  (the per-j factor cancels in the
  column softmax), so no second score/exp pass is needed:
  - E1t (i on partitions) comes from two XBAR DMA transposes (bf16).
  - CTu = Ct * u[i] via PE transposes + per-partition-scaled copies.
  - T2[j,d] = sum_i (E1*u)[i,j] Ct[i,d] accumulated directly with
    j on partitions; normalized by 1/s2sum via per-partition scale.
  - A' = QT-contract, B' = T-contract of E1^T; 1/s1sum broadcast by
    rank-1 matmuls of the reciprocal row.
  - 3-stage software pipeline (front1 | back(prev) | front2) so no
    engine queues batch b's early work behind batch b-1's tail;
    input DMAs ride the ACT hwdge queue, XBARs + out DMA ride SP.
  - reps>1 wraps the whole batch loop in a hardware For_i loop
    (used by bench to amortize the per-dispatch host overhead).

Data parallel over batch: 64 batches -> 8 NeuronCores x 8 batches.
Masks/w are pre-laid-out host-side (partition-major) in kernel().
"""

import os
from contextlib import ExitStack

import numpy as np

import concourse.bacc as bacc
import concourse.bass as bass
import concourse.tile as tile
from concourse import mybir
from concourse.masks import make_identity

B, D, LC, LQ = 64, 128, 1024, 256
NCORES = 8
BPC = B // NCORES  # batches per core

F32 = mybir.dt.float32
BF16 = mybir.dt.bfloat16
R = mybir.dt.float32r
AF = mybir.ActivationFunctionType
ALU = mybir.AluOpType

_CACHE: dict = {}


def _emit(nc: bass.Bass, tc, C_h, Q_h, cb_h, qb_h, w_h, out_h, reps: int):
    with ExitStack() as ctx:
        consts = ctx.enter_context(tc.tile_pool(name="consts", bufs=1))
        sb = ctx.enter_context(tc.tile_pool(name="sb", bufs=3))
        sb3 = ctx.enter_context(tc.tile_pool(name="sb3", bufs=3))
        ps_sc = ctx.enter_context(tc.tile_pool(name="ps_sc", bufs=2, space="PSUM"))
        ps_f2 = ctx.enter_context(tc.tile_pool(name="ps_f2", bufs=2, space="PSUM"))
        ps_tr = ctx.enter_context(tc.tile_pool(name="ps_tr", bufs=1, space="PSUM"))
        ps_bk = ctx.enter_context(tc.tile_pool(name="ps_bk", bufs=3, space="PSUM"))

        ident_f = consts.tile([128, 128], F32)
        make_identity(nc, ident_f[:])
        ident = consts.tile([128, 128], BF16)
        nc.vector.tensor_copy(ident[:], ident_f[:])
        ones_col = consts.tile([128, 1], BF16)
        nc.vector.memset(ones_col[:], 1.0)
        ones_row = consts.tile([1, 128], BF16)
        nc.vector.memset(ones_row[:], 1.0)

        # host-preprocessed: w as (p, k) cols [w1 w2 w3]; masks as biases
        w_sb = consts.tile([128, 3], F32)
        nc.sync.dma_start(out=w_sb[:], in_=w_h.ap())
        w_bf = consts.tile([128, 2], BF16)
        nc.vector.tensor_copy(w_bf[:], w_sb[:, 0:2])
        cb_sb = consts.tile([128, BPC, 8], F32)  # (cmask-1)*1e30, (p, b, t)
        nc.sync.dma_start(out=cb_sb[:], in_=cb_h.ap())
        qb_sb = consts.tile([128, BPC, 2], F32)  # (qmask-1)*1e30, (p, b, t)
        nc.sync.dma_start(out=qb_sb[:], in_=qb_h.ap())

        # 3-stage software pipeline state
        st1: list = [None] * BPC
        st2: list = [None] * BPC

        def front1(b):
            """DMA in, biases, scores+exp, QT."""
            ob = out_h.ap()[b]
            # input DMAs ride the ACT hwdge queue: they precede ACT's own
            # dependent ops (Qbf/Cbf/exps), so the ACT queue never blocks
            # across batches; SP keeps the XBARs + out DMA.
            # block 1 of the output is Ct itself: HBM->HBM, no SBUF dep
            nc.scalar.dma_start(out=ob[0:128, :], in_=C_h.ap()[b])
            C_sb = sb.tile([128, LC], F32, tag="C_sb")
            nc.scalar.dma_start(out=C_sb[:], in_=C_h.ap()[b])
            Q_sb = sb3.tile([128, LQ], F32, tag="Q_sb")
            nc.scalar.dma_start(out=Q_sb[:], in_=Q_h.ap()[b])

            Cw3 = sb.tile([128, LC], BF16, tag="Cw3")
            nc.vector.tensor_scalar_mul(Cw3[:], C_sb[:], w_sb[:, 2:3])
            Qbf = sb3.tile([128, LQ], BF16, tag="Qbf")
            nc.scalar.copy(out=Qbf[:], in_=Q_sb[:])
            Cbf = sb.tile([128, LC], BF16, tag="Cbf")
            nc.scalar.copy(out=Cbf[:], in_=C_sb[:])

            # r1/r2 biases; u = exp(r1+cb) per-partition cols
            rt = ps_sc.tile([128, 512], F32, tag="ps_sc")
            rall = rt[:, 0:16]
            r2p = rt[:, 32:34]
            for it in range(8):
                nc.tensor.matmul(rall[:, 2 * it:2 * it + 2],
                                 Cbf[:, it * 128:(it + 1) * 128],
                                 w_bf[:, 0:2], start=True, stop=True)
            for jt in range(2):
                nc.tensor.matmul(r2p[:, jt:jt + 1],
                                 Qbf[:, jt * 128:(jt + 1) * 128],
                                 w_bf[:, 1:2], start=True, stop=True)
            cbias = sb3.tile([128, 8], F32, tag="cbias")
            nc.vector.tensor_add(
                cbias[:],
                rall[:].rearrange("p (k two) -> p k two", two=2)[:, :, 0],
                cb_sb[:, b, :],
            )
            u_sb = sb3.tile([128, 8], F32, tag="u_sb")
            nc.scalar.activation(out=u_sb[:], in_=cbias[:], func=AF.Exp,
                                 bias=0.0, scale=1.0)
            u_bf = sb3.tile([128, 8], BF16, tag="u_bf")
            nc.vector.tensor_copy(u_bf[:], u_sb[:])
            bias1 = sb3.tile([128, 2], F32, tag="bias1")
            nc.vector.tensor_add(bias1[:], r2p[:], qb_sb[:, b, :])

            # QT (j on partitions, d free), bf16
            QT = sb3.tile([128, 256], BF16, tag="QT")
            pq = ps_tr.tile([128, 256], BF16, tag="ps_tr")
            for jt in range(2):
                nc.tensor.transpose(pq[:, jt * 128:(jt + 1) * 128],
                                    Qbf[:, jt * 128:(jt + 1) * 128],
                                    ident[:])
            nc.scalar.copy(out=QT[:], in_=pq[:])

            # E1^T = exp(S^T + r2[j] + qb[j]) bf16, j on partitions
            E1T = [sb.tile([128, LC], BF16, tag=f"E1T{jt}",
                           name=f"E1T{jt}") for jt in range(2)]
            for jt in range(2):
                for ic in range(2):
                    psc = ps_sc.tile([128, 512], F32, tag="ps_sc")
                    nc.tensor.matmul(psc[:],
                                     Qbf[:, jt * 128:(jt + 1) * 128],
                                     Cw3[:, ic * 512:(ic + 1) * 512],
                                     start=True, stop=True)
                    nc.scalar.activation(
                        out=E1T[jt][:, ic * 512:(ic + 1) * 512], in_=psc[:],
                        func=AF.Exp, bias=bias1[:, jt:jt + 1], scale=1.0)

            st1[b] = (C_sb, Cbf, u_sb, u_bf, E1T, QT, ob)

        def front2(b):
            """XBAR transposes, CTu, bc broadcast, Cbc."""
            (C_sb, Cbf, u_sb, u_bf, E1T, QT, ob) = st1[b]

            # E1t[p, (jt,t), jp] = E1T[jt][jp, t*128+p] via DMA XBAR
            E1t = sb.tile([128, 16, 128], BF16, tag="E1t")
            nc.scalar.dma_start_transpose(out=E1t[:, 0:8, :], in_=E1T[0][:])
            nc.scalar.dma_start_transpose(out=E1t[:, 8:16, :], in_=E1T[1][:])

            # CTu[i, (t,d)] = Ct * u[i] via PE transpose + scaled copy
            CTu = sb.tile([128, LC], BF16, tag="CTu")
            for g in range(4):
                ptc = ps_tr.tile([128, 256], BF16, tag="ps_tr")
                for k in range(2):
                    it = 2 * g + k
                    nc.tensor.transpose(ptc[:, k * 128:(k + 1) * 128],
                                        Cbf[:, it * 128:(it + 1) * 128],
                                        ident[:])
                for k in range(2):
                    it = 2 * g + k
                    nc.vector.tensor_scalar_mul(
                        CTu[:, it * 128:(it + 1) * 128],
                        ptc[:, k * 128:(k + 1) * 128],
                        u_sb[:, it:it + 1])

            # bc[i] = 1/s1sum broadcast to (128, LC); Cbc = C * bc
            rec_row = sb3.tile([1, LC], BF16, tag="rec_row")
            bc_sb = sb.tile([128, LC], F32, tag="bc_sb")
            for ic in range(2):
                s1t = ps_f2.tile([128, 512], F32, tag="ps_f2")
                s1p = s1t[0:1, 0:512]
                for jt in range(2):
                    nc.tensor.matmul(s1p[:], ones_col[:],
                                     E1T[jt][:, ic * 512:(ic + 1) * 512],
                                     start=(jt == 0), stop=(jt == 1))
                with nc.allow_low_precision(reason="bf16 norm row"):
                    nc.vector.reciprocal(
                        rec_row[0:1, ic * 512:(ic + 1) * 512], s1p[:])
                pbc = ps_f2.tile([128, 512], F32, tag="ps_f2")
                nc.tensor.matmul(pbc[:], ones_row[:],
                                 rec_row[0:1, ic * 512:(ic + 1) * 512],
                                 start=True, stop=True)
                nc.vector.tensor_scalar_mul(
                    bc_sb[:, ic * 512:(ic + 1) * 512], pbc[:], 1.0)
            Cbc = sb.tile([128, LC], F32, tag="Cbc")
            nc.gpsimd.tensor_mul(Cbc[:], C_sb[:], bc_sb[:])

            st2[b] = (u_bf, E1T, E1t, CTu, bc_sb, Cbc, Cbf, QT, ob)

        def back(b):
            """T path, A'/B', output blocks, out DMA."""
            (u_bf, E1T, E1t, CTu, bc_sb, Cbc, Cbf, QT, ob) = st2[b]
            st1[b] = None
            st2[b] = None
            E1tv = E1t[:]

            # A' and blk1/blk2 first: independent of the T path, overlaps
            # the T'2 accumulation chain below
            blkA = sb.tile([128, 3 * LC], F32, tag="blkA")
            blk1 = blkA[:, 0:LC]
            blk2 = blkA[:, LC:2 * LC]
            blk3 = blkA[:, 2 * LC:3 * LC]
            for ic in range(2):
                pA = ps_bk.tile([128, 512], F32, tag="ps_bk")
                for jt in range(2):
                    nc.tensor.matmul(pA[:], QT[:, jt * 128:(jt + 1) * 128],
                                     E1T[jt][:, ic * 512:(ic + 1) * 512],
                                     start=(jt == 0), stop=(jt == 1))
                nc.vector.tensor_mul(blk1[:, ic * 512:(ic + 1) * 512],
                                     pA[:],
                                     bc_sb[:, ic * 512:(ic + 1) * 512])
                nc.gpsimd.tensor_mul(blk2[:, ic * 512:(ic + 1) * 512],
                                     blk1[:, ic * 512:(ic + 1) * 512],
                                     Cbf[:, ic * 512:(ic + 1) * 512])

            # s2sum row (u-weighted col sums of E1); s2c in same tile
            s2t = ps_bk.tile([128, 512], F32, tag="ps_bk")
            s2row = s2t[0:1, 0:256]
            s2c = s2t[:, 256:258]
            for t in range(8):
                mv = E1tv[:, t::8, :]
                nc.tensor.matmul(s2row[:], u_bf[:, t:t + 1], mv,
                                 start=(t == 0), stop=(t == 7))
            s2rs = sb3.tile([1, 256], F32, tag="s2rs")
            nc.scalar.copy(out=s2rs[:], in_=s2row[:])

            # T2[j, d] = sum_i (E1*u)[i, j] * Ct[i, d] directly (j on parts)
            pT3 = ps_bk.tile([128, 512], F32, tag="ps_bk")
            for jh in range(2):
                for t in range(8):
                    nc.tensor.matmul(
                        pT3[:, jh * 128:(jh + 1) * 128],
                        E1tv[:, jh * 8 + t, :],
                        CTu[:, t * 128:(t + 1) * 128],
                        start=(t == 0), stop=(t == 7))

            # rec2 = 1/s2sum as columns; T_sb = T2 * rec2 (per-partition j)
            for jh in range(2):
                nc.tensor.transpose(s2c[:, jh:jh + 1],
                                    s2rs[0:1, jh * 128:(jh + 1) * 128],
                                    ident_f[0:1, 0:1])
            rec2 = sb3.tile([128, 2], F32, tag="rec2")
            nc.vector.reciprocal(rec2[:], s2c[:])
            T_sb = sb3.tile([128, 256], BF16, tag="T_sb")
            for jh in range(2):
                nc.scalar.activation(
                    out=T_sb[:, jh * 128:(jh + 1) * 128],
                    in_=pT3[:, jh * 128:(jh + 1) * 128], func=AF.Copy,
                    bias=0.0, scale=rec2[:, jh:jh + 1])

            # B' and blk3
            for ic in range(2):
                pB = ps_bk.tile([128, 512], F32, tag="ps_bk")
                for jt in range(2):
                    nc.tensor.matmul(pB[:],
                                     T_sb[:, jt * 128:(jt + 1) * 128],
                                     E1T[jt][:, ic * 512:(ic + 1) * 512],
                                     start=(jt == 0), stop=(jt == 1))
                nc.vector.tensor_mul(blk3[:, ic * 512:(ic + 1) * 512],
                                     pB[:],
                                     Cbc[:, ic * 512:(ic + 1) * 512])

            nc.sync.dma_start(
                out=ob[128:512, :].rearrange("(k p) i -> p k i", k=3),
                in_=blkA[:].rearrange("p (k i) -> p k i", k=3),
            )

        rep_i = tc.For_i(0, reps, 1)
        with rep_i:
            # 3-stage software pipeline, order per slot:
            #   front1(b), back(b-1), front2(b)
            # so the out-DMA of b-1 precedes the XBAR transposes of b on the
            # DMA queue, and no engine queues early work of b behind the
            # tail of b-1.
            for b in range(BPC):
                front1(b)
                if b > 0:
                    back(b - 1)
                front2(b)
            back(BPC - 1)


def build_nc(reps: int = 1) -> bass.Bass:
    nc = bacc.Bacc("TRN2", target_bir_lowering=False, debug=False)
    C_h = nc.dram_tensor("C", [BPC, D, LC], F32, kind="ExternalInput")
    Q_h = nc.dram_tensor("Q", [BPC, D, LQ], F32, kind="ExternalInput")
    cb_h = nc.dram_tensor("cbias", [128, BPC, 8], F32, kind="ExternalInput")
    qb_h = nc.dram_tensor("qbias", [128, BPC, 2], F32, kind="ExternalInput")
    w_h = nc.dram_tensor("w", [128, 3], F32, kind="ExternalInput")
    out_h = nc.dram_tensor("out", [BPC, 4 * D, LC], F32, kind="ExternalOutput")
    with tile.TileContext(nc) as tc:
        _emit(nc, tc, C_h, Q_h, cb_h, qb_h, w_h, out_h, reps)
    nc.compile()
    return nc


def _make_runner(nc):
    """Cached jitted SPMD executor (mirrors bass2jax.run_bass_via_pjrt)."""
    import jax
    from jax.experimental.shard_map import shard_map
    from jax.sharding import Mesh, PartitionSpec
    from concourse import bass2jax
    from concourse import mybir as _mb

    bass2jax.install_neuronx_cc_hook()
    partition_name = nc.partition_id_tensor.name if nc.partition_id_tensor else None
    in_names, out_names, out_avals = [], [], []
    for alloc in nc.m.functions[0].allocations:
        if not isinstance(alloc, _mb.MemoryLocationSet):
            continue
        name = alloc.memorylocations[0].name
        if alloc.kind == "ExternalInput":
            if name != partition_name:
                in_names.append(name)
        elif alloc.kind == "ExternalOutput":
            shape = tuple(alloc.tensor_shape)
            dtype = _mb.dt.np(alloc.dtype)
            out_names.append(name)
            out_avals.append(jax.core.ShapedArray(shape, dtype))
    n_params = len(in_names)
    n_outs = len(out_names)
    all_names = in_names + out_names + ([partition_name] if partition_name else [])

    def _body(*args):
        operands = list(args)
        if partition_name is not None:
            operands.append(bass2jax.partition_id_tensor())
        outs = bass2jax._bass_exec_p.bind(
            *operands,
            out_avals=tuple(out_avals),
            in_names=tuple(all_names),
            out_names=tuple(out_names),
            lowering_input_output_aliases=(),
            sim_require_finite=True,
            sim_require_nnan=True,
            nc=nc,
        )
        return tuple(outs)

    devices = jax.devices()[:NCORES]
    assert len(devices) == NCORES
    mesh = Mesh(np.asarray(devices), ("core",))
    in_specs = (PartitionSpec("core"),) * (n_params + n_outs)
    out_specs = (PartitionSpec("core"),) * n_outs
    donate = tuple(range(n_params, n_params + n_outs))
    fn = jax.jit(
        shard_map(
            _body, mesh=mesh, in_specs=in_specs, out_specs=out_specs, check_rep=False
        ),
        donate_argnums=donate,
        keep_unused=True,
    )
    return fn, in_names[:n_params], out_names, mesh


def _get_runner(reps: int = 1):
    key = f"runner{reps}"
    if key not in _CACHE:
        _CACHE[key] = _make_runner(build_nc(reps))
    return _CACHE[key]


def _global_args(C, Q, cmask, qmask, w, in_names):
    # host-side layout prep (small tensors): per-core partition-major
    # masks as additive biases, w as (p, k) columns.
    cb = ((cmask - 1.0) * 1e30).reshape(NCORES, BPC, 8, 128)
    cb = np.ascontiguousarray(cb.transpose(0, 3, 1, 2)).reshape(
        NCORES * 128, BPC, 8)
    qb = ((qmask - 1.0) * 1e30).reshape(NCORES, BPC, 2, 128)
    qb = np.ascontiguousarray(qb.transpose(0, 3, 1, 2)).reshape(
        NCORES * 128, BPC, 2)
    wk = np.ascontiguousarray(w.reshape(3, 128).T)  # (128, 3)
    wg = np.concatenate([wk] * NCORES, axis=0)
    vals = {"C": C, "Q": Q, "cbias": cb, "qbias": qb, "w": wg}
    return [vals[n] for n in in_names]


def kernel(C, Q, cmask, qmask, w):
    C = np.ascontiguousarray(np.asarray(C, dtype=np.float32))
    Q = np.ascontiguousarray(np.asarray(Q, dtype=np.float32))
    cmask = np.ascontiguousarray(np.asarray(cmask, dtype=np.float32))
    qmask = np.ascontiguousarray(np.asarray(qmask, dtype=np.float32))
    w = np.ascontiguousarray(np.asarray(w, dtype=np.float32))

    fn, in_names, out_names, mesh = _get_runner(1)
    args = _global_args(C, Q, cmask, qmask, w, in_names)
    donor = np.zeros((B, 4 * D, LC), np.float32)
    outs = fn(*args, donor)
    return np.asarray(outs[0]).astype(np.float32)


def bench(C, Q, cmask, qmask, w, iters=10, warmup=2, reps=None):
    """Per-iteration device time; launch overhead amortized over in-kernel
    hardware-loop reps (the reported time is wall/reps, an upper bound on
    the true per-iteration kernel time)."""
    import time as _time
    import jax
    from jax.sharding import NamedSharding, PartitionSpec

    if reps is None:
        reps = int(os.environ.get("CQA_REPS", "64"))
    fn, in_names, out_names, mesh = _get_runner(reps)
    sh = NamedSharding(mesh, PartitionSpec("core"))
    args = [jax.device_put(a, sh) for a in _global_args(
        np.ascontiguousarray(C, np.float32), np.ascontiguousarray(Q, np.float32),
        np.ascontiguousarray(cmask, np.float32),
        np.ascontiguousarray(qmask, np.float32),
        np.ascontiguousarray(w, np.float32), in_names)]
    out = jax.device_put(np.zeros((B, 4 * D, LC), np.float32), sh)
    for _ in range(warmup):
        out = fn(*args, out)[0]
    out.block_until_ready()
    t0 = _time.perf_counter()
    for _ in range(iters):
        out = fn(*args, out)[0]
    out.block_until_ready()
    t1 = _time.perf_counter()
    return (t1 - t0) / (iters * reps), np.asarray(out)
